# revision 1
# baseline (speedup 1.0000x reference)
"""Fused attention block (LGHIFusion) for Trainium2, 8-core tensor-parallel.

Math (per reference):
  Q = low  @ W_Q.T + b_Q ; K = low @ W_K.T + b_K ; V = high @ W_V.T + b_V
  attn = softmax(Q K^T / sqrt(dh)) ; ctx = attn @ V
  Z = ctx @ W_O.T + b_O ; out = low + sigmoid(gamma) * Z

Sharding: tensor-parallel over heads. 16 heads / 8 cores = 2 heads/core.
Each core computes QT/KT/VT for its 128 output dims, per-head attention
with scores kept TRANSPOSED ([k, q] layout) so softmax denominators come
free from an appended ones-column in V (no PE transposes of P needed),
then its partial Z = ctx @ W_O[:, shard].T (full 1024 output dims).
Host sums the 8 fp16 partials and applies residual + beta*b_O.

All matmuls run in bf16 (full PE rate, FWL weight loads, half DMA);
the beta=sigmoid(-5)~0.0067 gate damps kernel error by ~150x in the
final output, so bf16/fp16-partials error is small end to end.
"""

import numpy as np

try:
    import concourse.bass as bass
except ImportError:  # pragma: no cover
    import sys

    sys.path.insert(0, "/opt/trn_rl_repo")
    import concourse.bass as bass

import concourse.mybir as mybir
from concourse.bass_utils import run_bass_kernel_spmd
from concourse.masks import make_identity
from concourse.tile import TileContext

dt = mybir.dt
F32, BF16, F16 = dt.float32, dt.bfloat16, dt.float16
AF = mybir.ActivationFunctionType

B, S, D = 2, 2048, 1024
H, DH = 16, 64
T = B * S            # 4096 tokens
NCORES = 8
HPC = H // NCORES    # 2 heads per core
OPC = HPC * DH       # 128 out dims per core
VW = DH + 1          # V columns + ones column = 65
KT_N = S // 128      # 16 k-tiles per batch
NKT = T // 128       # 32 global token tiles
PCH = 512            # projection token-chunk size
QC = 1024            # q-chunk for attention


def _build_nc():
    nc = bass.Bass("TRN2", target_bir_lowering=False, debug=False,
                   num_devices=NCORES)

    xt_lo = nc.dram_tensor("xt_lo", [D, T], BF16, kind="ExternalInput").ap()
    xt_hi = nc.dram_tensor("xt_hi", [D, T], BF16, kind="ExternalInput").ap()
    wq_t = nc.dram_tensor("wq_t", [D, OPC], BF16, kind="ExternalInput").ap()
    wk_t = nc.dram_tensor("wk_t", [D, OPC], BF16, kind="ExternalInput").ap()
    wv_t = nc.dram_tensor("wv_t", [D, OPC], BF16, kind="ExternalInput").ap()
    wo_t = nc.dram_tensor("wo_t", [OPC, D], BF16, kind="ExternalInput").ap()
    bq_d = nc.dram_tensor("bq", [1, OPC], BF16, kind="ExternalInput").ap()
    bk_d = nc.dram_tensor("bk", [1, OPC], BF16, kind="ExternalInput").ap()
    bv_d = nc.dram_tensor("bv", [1, OPC], BF16, kind="ExternalInput").ap()
    z_out = nc.dram_tensor("z_out", [T, D], F16, kind="ExternalOutput").ap()

    with TileContext(nc) as tc:
        with (
            tc.tile_pool(name="const", bufs=1) as const,
            tc.tile_pool(name="w", bufs=1) as wpool,
            tc.tile_pool(name="x", bufs=2) as xpool,
            tc.tile_pool(name="acts", bufs=1) as actpool,
            tc.tile_pool(name="vone", bufs=1) as vpool,
            tc.tile_pool(name="pt", bufs=3) as ptpool,
            tc.tile_pool(name="ctxn", bufs=2) as cxpool,
            tc.tile_pool(name="z16", bufs=3) as zpool,
            tc.tile_pool(name="r", bufs=2) as rpool,
            tc.tile_pool(name="ps", bufs=2, space="PSUM") as pp,
            tc.tile_pool(name="pc", bufs=1, space="PSUM") as pc,
        ):
            ident = const.tile([128, 128], BF16)
            make_identity(nc, ident[:])

            wq = wpool.tile([128, D], BF16, tag="wq")
            wk = wpool.tile([128, D], BF16, tag="wk")
            wv = wpool.tile([128, D], BF16, tag="wv")
            wo = wpool.tile([128, D], BF16, tag="wo")
            for k in range(D // 128):
                nc.sync.dma_start(wq[:, 128 * k:128 * (k + 1)],
                                  wq_t[128 * k:128 * (k + 1), :])
                nc.sync.dma_start(wk[:, 128 * k:128 * (k + 1)],
                                  wk_t[128 * k:128 * (k + 1), :])
                nc.sync.dma_start(wv[:, 128 * k:128 * (k + 1)],
                                  wv_t[128 * k:128 * (k + 1), :])
            nc.sync.dma_start(wo[:], wo_t[:, :])
            bq = const.tile([1, OPC], BF16, tag="bq")
            bk = const.tile([1, OPC], BF16, tag="bk")
            bv = const.tile([1, OPC], BF16, tag="bv")
            nc.sync.dma_start(bq[:], bq_d[:, :])
            nc.sync.dma_start(bk[:], bk_d[:, :])
            nc.sync.dma_start(bv[:], bv_d[:, :])
            ones_p = const.tile([1, PCH], BF16, tag="ones_p")
            nc.vector.memset(ones_p[:], 1.0)
            ones64 = const.tile([1, DH], F32, tag="ones64")
            nc.vector.memset(ones64[:], 1.0)

            # Persistent activations: [128 outdims, token] transposed layout.
            qt = actpool.tile([128, T], BF16, tag="qt")
            kts = actpool.tile([128, T], BF16, tag="kt")
            vts = actpool.tile([128, T], BF16, tag="vt")
            # V in [k, dh] layout + ones column per (ktile, head).
            vone = vpool.tile([128, NKT * HPC * VW], BF16)
            nc.vector.memset(vone[:], 1.0)

            # ---- Phase B: projections (QT/KT/VT), streamed over tokens ----
            nd = D // 128
            for tch in range(T // PCH):
                t0 = tch * PCH
                xlo = xpool.tile([128, nd * PCH], BF16, tag="xlo")
                xhi = xpool.tile([128, nd * PCH], BF16, tag="xhi")
                for k in range(nd):
                    nc.sync.dma_start(xlo[:, PCH * k:PCH * (k + 1)],
                                      xt_lo[128 * k:128 * (k + 1), t0:t0 + PCH])
                    nc.sync.dma_start(xhi[:, PCH * k:PCH * (k + 1)],
                                      xt_hi[128 * k:128 * (k + 1), t0:t0 + PCH])
                for wmat, bias, dest, src in (
                    (wq, bq, qt, xlo),
                    (wk, bk, kts, xlo),
                    (wv, bv, vts, xhi),
                ):
                    ps = pp.tile([128, PCH], F32, tag="s")
                    for k in range(nd):
                        nc.tensor.matmul(
                            ps[:],
                            lhsT=wmat[:, 128 * k:128 * (k + 1)],
                            rhs=src[:, PCH * k:PCH * (k + 1)],
                            start=(k == 0), stop=False)
                    nc.tensor.matmul(ps[:], lhsT=bias[:], rhs=ones_p[:],
                                     start=False, stop=True)
                    nc.vector.tensor_copy(dest[:, t0:t0 + PCH], ps[:])

            # ---- Phase C: V -> [k, dh] via PE transpose, into vone ----
            for g in range(NKT):
                pt_ps = pc.tile([128, 128], BF16, tag="c")
                nc.tensor.transpose(pt_ps[:], vts[:, 128 * g:128 * (g + 1)],
                                    ident[:])
                for h in range(HPC):
                    base = (g * HPC + h) * VW
                    nc.vector.tensor_copy(vone[:, base:base + DH],
                                          pt_ps[:, DH * h:DH * (h + 1)])

            # ---- Phase D: attention, scores transposed [k, q] ----
            for b in range(B):
                ctxn = cxpool.tile([128, S], BF16)
                for h in range(HPC):
                    hp = DH * h
                    for qc in range(S // QC):
                        q0 = b * S + qc * QC
                        ps_c = pc.tile([VW, QC], F32, tag="c")
                        for kt in range(KT_N):
                            g = b * KT_N + kt
                            ps_s = pp.tile([128, QC], F32, tag="s")
                            for hf in range(QC // 512):
                                nc.tensor.matmul(
                                    ps_s[:, 512 * hf:512 * (hf + 1)],
                                    lhsT=kts[hp:hp + DH,
                                                   128 * g:128 * (g + 1)],
                                    rhs=qt[hp:hp + DH,
                                                 q0 + 512 * hf:
                                                 q0 + 512 * (hf + 1)],
                                    start=True, stop=True)
                            pt = ptpool.tile([128, QC], BF16)
                            nc.scalar.activation(pt[:], ps_s[:], AF.Exp,
                                                 scale=0.125)
                            vbase = (g * HPC + h) * VW
                            for hf in range(QC // 512):
                                nc.tensor.matmul(
                                    ps_c[:, 512 * hf:512 * (hf + 1)],
                                    lhsT=vone[:, vbase:vbase + VW],
                                    rhs=pt[:, 512 * hf:512 * (hf + 1)],
                                    start=(kt == 0), stop=(kt == KT_N - 1))
                        recip = rpool.tile([1, QC], F32, tag="recip")
                        nc.vector.reciprocal(recip[:], ps_c[DH:DH + 1, :])
                        ps_bc = pc.tile([DH, QC], F32, tag="bc")
                        for hf in range(QC // 512):
                            nc.tensor.matmul(
                                ps_bc[:, 512 * hf:512 * (hf + 1)],
                                lhsT=ones64[:],
                                rhs=recip[:, 512 * hf:512 * (hf + 1)],
                                start=True, stop=True)
                        bc_sb = rpool.tile([DH, QC], F32, tag="bc")
                        nc.vector.tensor_copy(bc_sb[:], ps_bc[:])
                        nc.vector.tensor_mul(
                            ctxn[hp:hp + DH, qc * QC:(qc + 1) * QC],
                            ps_c[0:DH, :], bc_sb[:])

                # ---- Phase E: partial Z = ctxN.T @ W_O_shard.T ----
                for qt_i in range(S // 128):
                    ps_z = pp.tile([128, D], F32, tag="s")
                    for hf in range(D // 512):
                        nc.tensor.matmul(
                            ps_z[:, 512 * hf:512 * (hf + 1)],
                            lhsT=ctxn[:, 128 * qt_i:128 * (qt_i + 1)],
                            rhs=wo[:, 512 * hf:512 * (hf + 1)],
                            start=True, stop=True)
                    z16 = zpool.tile([128, D], F16)
                    nc.vector.tensor_copy(z16[:], ps_z[:])
                    r0 = b * S + 128 * qt_i
                    nc.sync.dma_start(z_out[r0:r0 + 128, :], z16[:])

    _split_waits(nc)
    return nc


def _split_waits(nc):
    """This walrus build accepts only one sync-wait per instruction.
    Move extra waits onto same-engine NoOps inserted just before each
    offender (engine program order preserves the gating)."""
    for f in nc.m.functions:
        for blk in f.blocks:
            new_insts = []
            for inst in blk.instructions:
                si = inst.sync_info
                if si is not None and si.on_wait and len(si.on_wait) > 1:
                    waits = list(si.on_wait)
                    for w in waits[:-1]:
                        nop = mybir.InstNoOp(
                            name=nc.get_next_instruction_name(),
                            sync_info=mybir.SyncInfo(on_wait=[w],
                                                     on_update=[]),
                            bass_nofuse=True,
                            engine=inst.engine,
                        )
                        new_insts.append(nop)
                    si.on_wait = [waits[-1]]
                new_insts.append(inst)
            blk.instructions[:] = new_insts


_NC_CACHE = None


def _get_nc():
    global _NC_CACHE
    if _NC_CACHE is None:
        _NC_CACHE = _build_nc()
    return _NC_CACHE


def _make_in_maps(inputs):
    low = np.ascontiguousarray(np.asarray(inputs["low_freq"], np.float32))
    high = np.ascontiguousarray(np.asarray(inputs["high_freq"], np.float32))
    W_Q = np.asarray(inputs["W_Q"], np.float32)
    W_K = np.asarray(inputs["W_K"], np.float32)
    W_V = np.asarray(inputs["W_V"], np.float32)
    W_O = np.asarray(inputs["W_O"], np.float32)
    b_Q = np.asarray(inputs["b_Q"], np.float32)
    b_K = np.asarray(inputs["b_K"], np.float32)
    b_V = np.asarray(inputs["b_V"], np.float32)

    import ml_dtypes
    bf16 = ml_dtypes.bfloat16
    xt_lo = np.ascontiguousarray(low.reshape(T, D).T.astype(bf16))
    xt_hi = np.ascontiguousarray(high.reshape(T, D).T.astype(bf16))

    in_maps = []
    for c in range(NCORES):
        sl = slice(OPC * c, OPC * (c + 1))
        in_maps.append({
            "xt_lo": xt_lo,
            "xt_hi": xt_hi,
            "wq_t": np.ascontiguousarray(W_Q[sl, :].T.astype(bf16)),
            "wk_t": np.ascontiguousarray(W_K[sl, :].T.astype(bf16)),
            "wv_t": np.ascontiguousarray(W_V[sl, :].T.astype(bf16)),
            "wo_t": np.ascontiguousarray(W_O[:, sl].T.astype(bf16)),
            "bq": np.ascontiguousarray(b_Q[sl].reshape(1, OPC).astype(bf16)),
            "bk": np.ascontiguousarray(b_K[sl].reshape(1, OPC).astype(bf16)),
            "bv": np.ascontiguousarray(b_V[sl].reshape(1, OPC).astype(bf16)),
        })
    return in_maps


def _run(inputs, trace=False, **kw):
    low = np.ascontiguousarray(np.asarray(inputs["low_freq"], np.float32))
    b_O = np.asarray(inputs["b_O"], np.float32)
    gamma = float(np.asarray(inputs["gamma"], np.float32))
    in_maps = _make_in_maps(inputs)

    nc = _get_nc()
    res = run_bass_kernel_spmd(nc, in_maps, list(range(NCORES)), trace=trace,
                               **kw)

    zsum = np.zeros((T, D), np.float32)
    for r in res.results:
        zsum += r["z_out"].astype(np.float32)
    beta = 1.0 / (1.0 + np.exp(-gamma))
    out = low.reshape(T, D) + beta * (zsum + b_O[None, :])
    return out.reshape(B, S, D), res


def kernel(**inputs):
    out, _ = _run(inputs)
    return out



# revision 2
# speedup vs baseline: 1.0035x; 1.0035x over previous
"""Fused attention block (LGHIFusion) for Trainium2, 8-core tensor-parallel.

v3 (tuned for ~1.2 GHz sustained PE clock -> minimize PE cycles):
  fp8(e4m3) DoubleRow matmuls (2x contraction) for projections/PV/denoms;
  transposed scores [k, q] (psum-write bound, 1 cycle/col);
  denominators D via all-ones DoubleRow matmul (rows replicated free);
  1/D: PE chunk-transposes of D rows -> strided DVE reciprocal [128,16]
       -> PE transpose-back -> DRAM-bounce partition-flatten -> r rows;
  bc = ones64 x r broadcast matmul; ctxn_h = cxs_h * bc on DVE;
  ctxn_h1 partition-shifted 64->127 via SBUF-SBUF DMA so Z runs at full
  128-contraction; Z = ctxn.T @ W_O in bf16, plain f16 copies out.
  Biases folded into the projection PSUM->SBUF copies (per-partition
  scalar add on Act/DVE), not matmuls.
Host sums the 8 f16 partials and applies residual + beta*b_O.
"""

import numpy as np

try:
    import concourse.bass as bass
except ImportError:  # pragma: no cover
    import sys

    sys.path.insert(0, "/opt/trn_rl_repo")
    import concourse.bass as bass

import concourse.mybir as mybir
from concourse.bass_utils import run_bass_kernel_spmd
from concourse.masks import make_identity
from concourse.tile import TileContext

dt = mybir.dt
F32, BF16, F16, FP8 = dt.float32, dt.bfloat16, dt.float16, dt.float8e4
AF = mybir.ActivationFunctionType
PM = mybir.MatmulPerfMode
ALU = mybir.AluOpType

B, S, D = 2, 2048, 1024
H, DH = 16, 64
T = B * S
NCORES = 8
OPC = 128            # out dims per core (2 heads)
ND = D // 128        # 8 d-chunks
QC = 1024            # q chunk
NQC = S // QC        # 2 per batch
KT = S // 128        # 16 k-tiles per batch
NP = KT // 2         # 8 k-tile pairs
PCH = 512            # projection token chunk


def _build_nc():
    nc = bass.Bass("TRN2", target_bir_lowering=False, debug=False,
                   num_devices=NCORES)

    xt_lo = nc.dram_tensor("xt_lo", [D, T], FP8, kind="ExternalInput").ap()
    xt_hi = nc.dram_tensor("xt_hi", [D, T], FP8, kind="ExternalInput").ap()
    wq_d = nc.dram_tensor("wq", [D, OPC], FP8, kind="ExternalInput").ap()
    wk_d = nc.dram_tensor("wk", [D, OPC], FP8, kind="ExternalInput").ap()
    wv_d = nc.dram_tensor("wv", [D, OPC], FP8, kind="ExternalInput").ap()
    bq_d = nc.dram_tensor("bq", [64, 2], F32, kind="ExternalInput").ap()
    bk_d = nc.dram_tensor("bk", [64, 2], F32, kind="ExternalInput").ap()
    bv_d = nc.dram_tensor("bv", [64, 2], F32, kind="ExternalInput").ap()
    wo_d = nc.dram_tensor("wo", [OPC, D], BF16, kind="ExternalInput").ap()
    z_out = nc.dram_tensor("z_out", [T, D], F16, kind="ExternalOutput").ap()
    rhop = [nc.dram_tensor(f"rhop{i}", [16, 128], BF16,
                           kind="Internal").ap() for i in range(4)]

    with TileContext(nc) as tc:
        with (
            tc.tile_pool(name="const", bufs=1) as const,
            tc.tile_pool(name="acts", bufs=1) as acts,
            tc.tile_pool(name="work", bufs=2) as work,
            tc.tile_pool(name="scp", bufs=2, space="PSUM") as scp,
            tc.tile_pool(name="cxp", bufs=1, space="PSUM") as cxp,
            tc.tile_pool(name="dzp", bufs=1, space="PSUM") as dzp,
        ):
            # ---- constants / weights ----
            ident64 = const.tile([64, 64], BF16, tag="id64")
            make_identity(nc, ident64[:])
            ident32 = const.tile([32, 32], F32, tag="id32")
            make_identity(nc, ident32[:])
            ident128 = const.tile([128, 128], F32, tag="id128")
            make_identity(nc, ident128[:])
            ones2 = const.tile([128, 2, 64], FP8, tag="ones2")
            nc.vector.memset(ones2[:], 1.0)
            onescol = const.tile([1, 64], BF16, tag="onescol")
            nc.vector.memset(onescol[:], 1.0)

            wq = const.tile([128, ND, OPC], FP8, tag="wq")
            wk = const.tile([128, ND, OPC], FP8, tag="wk")
            wv = const.tile([128, ND, OPC], FP8, tag="wv")
            nc.sync.dma_start(
                wv[:], wv_d[:, :].rearrange("(c p) o -> p c o", p=128))
            bqs = const.tile([64, 2], F32, tag="bqs")
            bks = const.tile([64, 2], F32, tag="bks")
            bvs = const.tile([64, 2], F32, tag="bvs")
            nc.sync.dma_start(bqs[:], bq_d[:, :])
            nc.sync.dma_start(bks[:], bk_d[:, :])
            nc.sync.dma_start(bvs[:], bv_d[:, :])
            wo = const.tile([128, D], BF16, tag="wo")
            nc.sync.dma_start(wo[:], wo_d[:, :])

            # ---- x staged fully (fp8) ----
            xlo = const.tile([128, ND, T], FP8, tag="xlo")
            xhi = const.tile([128, ND, T], FP8, tag="xhi")
            def xdma(xt, xs, b, half):
                t0 = b * S + half * 1024
                nc.sync.dma_start(
                    xs[:, :, t0:t0 + 1024],
                    xt[:, t0:t0 + 1024].rearrange("(c p) t -> p c t",
                                                  p=128))
            xdma(xt_hi, xhi, 0, 0)
            nc.sync.dma_start(
                wq[:], wq_d[:, :].rearrange("(c p) o -> p c o", p=128))
            nc.sync.dma_start(
                wk[:], wk_d[:, :].rearrange("(c p) o -> p c o", p=128))
            xdma(xt_lo, xlo, 0, 0)
            xdma(xt_lo, xlo, 0, 1)
            xdma(xt_hi, xhi, 0, 1)
            xdma(xt_hi, xhi, 1, 0)
            xdma(xt_lo, xlo, 1, 0)
            xdma(xt_lo, xlo, 1, 1)
            xdma(xt_hi, xhi, 1, 1)

            # ---- persistent activations ----
            qt = acts.tile([64, 2, T], FP8, tag="qt")
            kt_ = acts.tile([64, 2, T], FP8, tag="kt")
            vts = acts.tile([64, 2, T], BF16, tag="vts")
            vone = [acts.tile([128, 2 * KT, DH], FP8, tag=f"vone{h}",
                              name=f"vone{h}") for h in range(2)]
            Pb = [acts.tile([128, KT, QC], FP8, tag=f"P{h}", name=f"P{h}")
                  for h in range(2)]

            # ---------- emission helpers ----------
            PROJ = {"q": (0, 1, 2), "k": (1,), "v": (2,)}

            def proj_chunk(b, tch, on_act, which="qkv"):
                """Project one 512-token chunk of batch b.
                Bias added during the PSUM->SBUF copy (per-partition)."""
                t0 = b * S + tch * PCH
                mats = (
                    (wq, bqs, qt, scp),
                    (wk, bks, kt_, scp),
                    (wv, bvs, vts, dzp),
                )
                sel = {"q": mats[0:1], "k": mats[1:2], "v": mats[2:3],
                       "qkv": mats, "qk": mats[0:2]}[which]
                for wt, bs, dest, pool in sel:
                    ps = pool.tile([64, 1024], F32,
                                   tag="s" if pool is scp else "d",
                                   name="pproj")
                    for h in range(2):
                        o = slice(64 * h, 64 * h + 64)
                        for c in range(4):
                            nc.tensor.matmul(
                                ps[:, 512 * h:512 * (h + 1)],
                                lhsT=wt[:, 2 * c:2 * c + 2, o],
                                rhs=(xhi if wt is wv else xlo)[
                                    :, 2 * c:2 * c + 2, t0:t0 + PCH],
                                start=(c == 0), stop=(c == 3),
                                perf_mode=PM.DoubleRow)
                    for h in range(2):
                        src = ps[:, 512 * h:512 * (h + 1)]
                        dst = dest[:, h:h + 1, t0:t0 + PCH]
                        if on_act:
                            nc.scalar.activation(dst, src, AF.Identity,
                                                 bias=bs[:, h:h + 1])
                        else:
                            nc.vector.tensor_scalar(dst, src, bs[:, h:h + 1],
                                                    None, ALU.add)

            def vtrans_group(b, h, g4):
                """Transpose 4 k-tiles of V for head h into vone (fp8)."""
                vtr = dzp.tile([128, 256], BF16, tag="d", name="vtr")
                for j in range(4):
                    gg = b * KT + g4 * 4 + j
                    src = vts[:, h:h + 1, 128 * gg:128 * (gg + 1)]
                    nc.tensor.transpose(vtr[:, 64 * j:64 * (j + 1)],
                                        src, ident64[:])
                g0 = b * KT + 4 * g4
                nc.vector.tensor_copy(vone[h][:, g0:g0 + 4, :], vtr[:])

            def make_z_thunks(b, qc, ctxn, final=False):
                """Z stream for (b, qc): full 128-contraction per q-tile."""
                thunks = []
                for qi in range(8):
                    def th(qi=qi, ctxn=ctxn, b=b, qc=qc):
                        if final and qi % 2 == 1:
                            zt = cxp.tile([128, 1024], F32, tag="c",
                                          name="ztc")
                        else:
                            zt = dzp.tile([128, 1024], F32, tag="d",
                                          name="zt")
                        for f in range(2):
                            nc.tensor.matmul(
                                zt[:, 512 * f:512 * (f + 1)],
                                lhsT=ctxn[:, 128 * qi:128 * (qi + 1)],
                                rhs=wo[:, 512 * f:512 * (f + 1)],
                                start=True, stop=True)
                        z16 = work.tile([128, 1024], F16, tag="z16",
                                        name="z16")
                        if qi % 2 == 0:
                            nc.vector.tensor_copy(z16[:], zt[:])
                        else:
                            nc.scalar.copy(z16[:], zt[:])
                        r0 = b * S + qc * QC + 128 * qi
                        nc.sync.dma_start(z_out[r0:r0 + 128, :], z16[:])
                    thunks.append(th)
                return thunks

            def attn_block(b, qc, filler, rhopd):
                """Attention for (b, qc): both heads; returns ctxn tile."""
                q0 = b * S + qc * QC
                ctxn = work.tile([128, QC], BF16, tag="ctxn", name="ctxn")
                cxs = [work.tile([64, QC], F32, tag=f"cxs{h}",
                                 name=f"cxs{h}") for h in range(2)]
                for h in range(2):
                    P = Pb[h]
                    cx = cxp.tile([64, QC], F32, tag="c", name="cx")
                    for kt in range(KT):
                        sct = scp.tile([128, QC], F32, tag="s", name="sct")
                        ks = b * KT + kt
                        for qh in range(2):
                            nc.tensor.matmul(
                                sct[:, 512 * qh:512 * (qh + 1)],
                                lhsT=kt_[:, h:h + 1,
                                         128 * ks:128 * (ks + 1)],
                                rhs=qt[:, h:h + 1,
                                       q0 + 512 * qh:q0 + 512 * (qh + 1)],
                                start=True, stop=True)
                        nc.scalar.activation(P[:, kt:kt + 1, :], sct[:],
                                             AF.Exp, scale=0.125)
                        if kt % 2 == 1:
                            p = kt // 2
                            g0 = b * KT + 2 * p
                            for qh in range(2):
                                nc.tensor.matmul(
                                    cx[:, 512 * qh:512 * (qh + 1)],
                                    lhsT=vone[h][:, g0:g0 + 2, :],
                                    rhs=P[:, 2 * p:2 * p + 2,
                                          512 * qh:512 * (qh + 1)],
                                    start=(p == 0), stop=(p == NP - 1),
                                    perf_mode=PM.DoubleRow)
                            npop = 2 if len(filler) > 6 else 1
                            for _ in range(npop):
                                if filler:
                                    filler.pop(0)()
                    # free the cx psum slot fast: stage to SBUF f32
                    nc.vector.tensor_copy(cxs[h][:], cx[:])

                # denominator rows per head; transposed reciprocal chain
                dns = work.tile([32, 2, QC], F32, tag="dns", name="dns")
                for h in range(2):
                    dn = dzp.tile([64, QC], F32, tag="d", name="dn")
                    for p in range(NP):
                        for qh in range(2):
                            nc.tensor.matmul(
                                dn[:, 512 * qh:512 * (qh + 1)],
                                lhsT=ones2[:],
                                rhs=Pb[h][:, 2 * p:2 * p + 2,
                                          512 * qh:512 * (qh + 1)],
                                start=(p == 0), stop=(p == NP - 1),
                                perf_mode=PM.DoubleRow)
                    nc.scalar.copy(dns[:, h:h + 1, :], dn[0:32, :])
                    if filler:
                        filler.pop(0)()
                dnT = scp.tile([128, 512], F32, tag="s", name="dnT")
                for h in range(2):
                    for c in range(8):
                        nc.tensor.transpose(
                            dnT[:, 256 * h + 32 * c:256 * h + 32 * (c + 1)],
                            dns[:, h:h + 1, 128 * c:128 * (c + 1)],
                            ident32[:])
                    if filler:
                        filler.pop(0)()
                rT = work.tile([128, 16], F32, tag="rt", name="rT")
                nc.vector.reciprocal(rT[:], dnT[:, 0:512:32])
                # transpose back -> [16, 128], DRAM bounce -> rows [1, 2048]
                rbp = scp.tile([16, 128], F32, tag="s", name="rbp")
                nc.tensor.transpose(rbp[:], rT[:], ident128[:])
                rbs = work.tile([16, 128], BF16, tag="rbs", name="rbs")
                nc.scalar.copy(rbs[:], rbp[:])
                for _ in range(2):
                    if filler:
                        filler.pop(0)()
                nc.sync.dma_start(rhopd[:, :], rbs[:])
                # broadcast 1/D rows to 64 partitions via stride-0 DMA
                bcs = work.tile([64, 2, QC], BF16, tag="bcs", name="bcs")
                for h in range(2):
                    for c in range(8):
                        nc.sync.dma_start(
                            bcs[:, h:h + 1, 128 * c:128 * (c + 1)],
                            rhopd[8 * h + c:8 * h + c + 1,
                                  :].broadcast_to((64, 128)))
                    if filler:
                        filler.pop(0)()
                # normalized ctxn: h0 -> rows 0:64; h1 -> tmp, DMA shift
                nc.vector.tensor_tensor(ctxn[0:64, :], cxs[0][:],
                                        bcs[:, 0:1, :], ALU.mult)
                ctmp = work.tile([64, QC], BF16, tag="ctmp", name="ctmp")
                nc.vector.tensor_tensor(ctmp[:], cxs[1][:],
                                        bcs[:, 1:2, :], ALU.mult)
                nc.sync.dma_start(ctxn[64:128, :], ctmp[:])
                return ctxn

            # ---------- schedule ----------
            # minimal head: only what (b0, qc0) kt0..3 needs
            proj_chunk(0, 0, on_act=True, which="v")
            vtrans_group(0, 0, 0)
            vtrans_group(0, 1, 0)
            proj_chunk(0, 0, on_act=True, which="qk")
            proj_chunk(0, 1, on_act=True, which="q")

            filler = []
            for tch in range(1, 4):
                filler.append(lambda tch=tch: proj_chunk(
                    0, tch, on_act=False, which="k"))
                filler.append(lambda tch=tch: proj_chunk(
                    0, tch, on_act=False, which="v"))
                filler.append(lambda tch=tch: vtrans_group(0, 0, tch))
                filler.append(lambda tch=tch: vtrans_group(0, 1, tch))
            for tch in range(2, 4):
                filler.append(lambda tch=tch: proj_chunk(
                    0, tch, on_act=False, which="q"))
            for tch in range(4):
                filler.append(lambda tch=tch: proj_chunk(
                    1, tch, on_act=False, which="v"))
                filler.append(lambda tch=tch: vtrans_group(1, 0, tch))
                filler.append(lambda tch=tch: vtrans_group(1, 1, tch))
            for tch in range(4):
                filler.append(lambda tch=tch: proj_chunk(
                    1, tch, on_act=False, which="q"))
                filler.append(lambda tch=tch: proj_chunk(
                    1, tch, on_act=False, which="k"))

            ctxn00 = attn_block(0, 0, filler, rhop[0])
            z00 = make_z_thunks(0, 0, ctxn00)
            filler = filler + z00
            ctxn01 = attn_block(0, 1, filler, rhop[1])
            z01 = make_z_thunks(0, 1, ctxn01)
            filler = filler + z01
            ctxn10 = attn_block(1, 0, filler, rhop[2])
            z10 = make_z_thunks(1, 0, ctxn10)
            filler = filler + z10
            ctxn11 = attn_block(1, 1, filler, rhop[3])
            while filler:
                filler.pop(0)()
            for th in make_z_thunks(1, 1, ctxn11, final=True):
                th()

    _split_waits(nc)
    return nc


def _split_waits(nc):
    """This walrus build accepts only one sync-wait per instruction.
    Move extra waits onto same-engine NoOps inserted just before each
    offender (engine program order preserves the gating)."""
    for f in nc.m.functions:
        for blk in f.blocks:
            new_insts = []
            for inst in blk.instructions:
                si = inst.sync_info
                if si is not None and si.on_wait and len(si.on_wait) > 1:
                    waits = list(si.on_wait)
                    for w in waits[:-1]:
                        nop = mybir.InstNoOp(
                            name=nc.get_next_instruction_name(),
                            sync_info=mybir.SyncInfo(on_wait=[w],
                                                     on_update=[]),
                            bass_nofuse=True,
                            engine=inst.engine,
                        )
                        new_insts.append(nop)
                    si.on_wait = [waits[-1]]
                new_insts.append(inst)
            blk.instructions[:] = new_insts


_NC_CACHE = None


def _get_nc():
    global _NC_CACHE
    if _NC_CACHE is None:
        _NC_CACHE = _build_nc()
    return _NC_CACHE


def _make_in_maps(inputs):
    import ml_dtypes
    fp8 = ml_dtypes.float8_e4m3fn
    bf16 = ml_dtypes.bfloat16

    low = np.ascontiguousarray(np.asarray(inputs["low_freq"], np.float32))
    high = np.ascontiguousarray(np.asarray(inputs["high_freq"], np.float32))
    W_Q = np.asarray(inputs["W_Q"], np.float32)
    W_K = np.asarray(inputs["W_K"], np.float32)
    W_V = np.asarray(inputs["W_V"], np.float32)
    W_O = np.asarray(inputs["W_O"], np.float32)
    b_Q = np.asarray(inputs["b_Q"], np.float32)
    b_K = np.asarray(inputs["b_K"], np.float32)
    b_V = np.asarray(inputs["b_V"], np.float32)

    xt_lo = np.ascontiguousarray(low.reshape(T, D).T.astype(fp8))
    xt_hi = np.ascontiguousarray(high.reshape(T, D).T.astype(fp8))

    in_maps = []
    for c in range(NCORES):
        sl = slice(OPC * c, OPC * (c + 1))
        in_maps.append({
            "xt_lo": xt_lo,
            "xt_hi": xt_hi,
            "wq": np.ascontiguousarray(W_Q[sl, :].T.astype(fp8)),
            "wk": np.ascontiguousarray(W_K[sl, :].T.astype(fp8)),
            "wv": np.ascontiguousarray(W_V[sl, :].T.astype(fp8)),
            "bq": np.ascontiguousarray(
                b_Q[sl].reshape(2, 64).T.astype(np.float32)),
            "bk": np.ascontiguousarray(
                b_K[sl].reshape(2, 64).T.astype(np.float32)),
            "bv": np.ascontiguousarray(
                b_V[sl].reshape(2, 64).T.astype(np.float32)),
            "wo": np.ascontiguousarray(W_O[:, sl].T.astype(bf16)),
        })
    return in_maps


def _run(inputs, trace=False, **kw):
    low = np.ascontiguousarray(np.asarray(inputs["low_freq"], np.float32))
    b_O = np.asarray(inputs["b_O"], np.float32)
    gamma = float(np.asarray(inputs["gamma"], np.float32))
    in_maps = _make_in_maps(inputs)

    nc = _get_nc()
    res = run_bass_kernel_spmd(nc, in_maps, list(range(NCORES)), trace=trace,
                               **kw)

    zsum = np.zeros((T, D), np.float32)
    for r in res.results:
        zsum += r["z_out"].astype(np.float32)
    beta = 1.0 / (1.0 + np.exp(-gamma))
    out = low.reshape(T, D) + beta * (zsum + b_O[None, :])
    return out.reshape(B, S, D), res


def kernel(**inputs):
    out, _ = _run(inputs)
    return out
